# revision 31
# baseline (speedup 1.0000x reference)
"""Triangular pairwise channel product on 8 Trainium2 NeuronCores.

out[b,h,w,k] = x[b,h,w,i_k] * x[b,h,w,j_k]  for the C*(C-1)/2 pairs
(i<j) in row-major (np.triu_indices) order.

Sharding: pure data parallel over batch - core c takes x[2c:2c+2].
Per core the 2*64*64 = 8192 spatial positions map to 128 SBUF
partitions (b_loc*64+h) x 64 groups (w).

d-offset formulation: for d in 1..63, prod_d[p,g,c] = x[p,g,c] *
x[p,g,c+d], c in [0, 64-d).  All operands are step-1 packed bf16 so
DVE's 2x_1p perf mode applies (2 elem/cycle).  Odd d reads its second
operand from x_odd (a one-channel-shifted DMA copy of x) to keep the
4B alignment 2x_1p requires.

v2 (stair-step): several consecutive same-parity d's share ONE
tensor_mul via a 4-level access pattern (partition, g, d, c): operand
a has d-stride 0 (broadcast), operand b d-stride 2 (overlapping
windows), out d-stride w_pad.  n=3 cuts DVE per-op overhead ~3x for
+6% rectangle-padding waste in SBUF/DMA bytes.  Measured baseline
facts: DVE tensor_tensor ~77ns/op marginal overhead, stores stream at
~433 GB/s/core on one HWDGE ring, ~7us fixed engine-barrier preamble,
~2.7us postamble.
"""

import numpy as np

import concourse.bacc as bacc
import concourse.bass as bass
import concourse.mybir as mybir
import concourse.tile as tile
from concourse.bass import AP
from concourse.bass_utils import run_bass_kernel_spmd

B, H, W, C = 16, 64, 64, 64
K = C * (C - 1) // 2  # 2016
N_CORES = 8
BP = B // N_CORES  # batch rows per core
P = BP * H         # 128 SBUF partitions
G = W              # position groups per partition
XLEN = G * C       # 4096 elements per partition
XPAD = 8           # tail pad: widest op row reads up to 4099
FP = mybir.dt.float32
BF = mybir.dt.bfloat16

# ---------------------------------------------------------------------------
# Stair-step plan: groups of n consecutive same-parity d's per DVE op.
# Group = (src_is_odd, d0, n, w_pad, qoff); row r handles d = d0 + 2r,
# covering c in [0, w_pad) (true width 64-d, rest is pad/waste).
# ---------------------------------------------------------------------------

def _make_plan(n: int = 3, extra_pad: int = 6):
    """extra_pad widens the last even group (single row d=62, w_pad 2)
    so K2 lands on a rounder number (2170 -> 2176): keeps the
    per-partition DRAM stride HBM-interleave friendly.  Its spill reads
    stay inside XPAD (62 + w_pad - 1 = 69 < 64 + XPAD)."""
    plan = []
    qoff = 0
    for parity in (0, 1):  # even d's first (they don't need x_odd)
        if parity == 0:
            ds = [d for d in range(2, C, 2)]
        else:
            ds = [d for d in range(1, C, 2)]
        i = 0
        while i < len(ds):
            grp = ds[i : i + n]
            w = C - grp[0]
            w_pad = w + (w % 2)
            if parity == 0 and i + n >= len(ds) and len(grp) == 1:
                w_pad += extra_pad
            plan.append((parity == 1, grp[0], len(grp), w_pad, qoff))
            qoff += len(grp) * w_pad
            i += n
    return plan, qoff


PLAN, K2 = _make_plan(3, extra_pad=0)
assert K2 == 2170, K2

# GpSimd offload was tried and regressed badly (one op took 45 us and
# concurrent SBUF traffic slowed DVE ~14%): engines fight over SBUF
# bandwidth.  Keep everything on DVE.
GP_GROUPS: set[int] = set()

# Ramped: small first tiles prime the store pipeline early (kills the
# DMA bubble waiting on a big tile-1 compute); small last tile cuts the
# post-DVE tail (its cast + fp8 store + postamble end the kernel).
G_ITERS = [2, 4, 8, 12, 12, 12, 11, 3]
assert sum(G_ITERS) == G

_CUM = [0]
for _o, _d0, _n, _wp, _qo in PLAN:
    _CUM.append(_CUM[-1] + _n * _wp)

# fp8 split: blocks [K2B, K2) (the odd-parity d>=13 groups, ~35% of
# channels) are cast bf16->fp8_e4m3 on the idle scalar engine and
# stored at half the bytes.  Combined rel err ~1.6e-2 vs the 2e-2
# gate (bf16-only is 2.9e-3).  Cuts store bytes 17%: stores are the
# bottleneck in good runs and doubly so when a DMA engine runs
# degraded (~21% slower, environmental).
K2B = _CUM[13]      # bf16 channels (1418); fp8 channels = K2 - K2B
K8 = K2 - K2B
# Sub-tile store points: ops 0..6 cover [0:918) >= [0:908); ops 0..12
# cover [0:K2B).  Store A (Q1) [0:908), store B (Q10) [908:K2B), then
# two fp8 casts (after op 16 and op 21) each followed by their store
# (Q10).  908 balances Q1 vs Q10 bytes (Q10 also carries the 1MB bulk
# loads).  K8M splits the casts so most fp8 bytes stream while the
# last DVE ops still run.
K2A = 908
K8M = _CUM[17] - K2B
assert _CUM[7] >= K2A

# Host-side permutation: out[g, k] (triu pair k) -> flat device position.
# Device layout per g-iteration chunk (base g_off*K2): ops write contiguous
# blocks [qoff*Gi, (qoff + n*w_pad)*Gi), each block laid out (g, r, c).
_II, _JJ = np.triu_indices(C, k=1)
_qoff_d = np.zeros(C, dtype=np.int64)
_r_d = np.zeros(C, dtype=np.int64)
_wpad_d = np.zeros(C, dtype=np.int64)
_blk_d = np.zeros(C, dtype=np.int64)  # n*w_pad of d's group
for _odd, _d0, _n, _wp, _qo in PLAN:
    for _r in range(_n):
        _d = _d0 + 2 * _r
        _qoff_d[_d] = _qo
        _r_d[_d] = _r
        _wpad_d[_d] = _wp
        _blk_d[_d] = _n * _wp
_D = _JJ - _II  # [K]
_g_off_g = np.zeros(G, dtype=np.int64)
_Gi_g = np.zeros(G, dtype=np.int64)
_go = 0
for _Gi in G_ITERS:
    _g_off_g[_go : _go + _Gi] = _go
    _Gi_g[_go : _go + _Gi] = _Gi
    _go += _Gi
# IDX[g, k]: position in the [G*K2] flat per-partition output vector.
_gg = np.arange(G, dtype=np.int64)[:, None]
_IDX = (
    _g_off_g[:, None] * K2
    + _qoff_d[_D][None, :] * _Gi_g[:, None]
    + (_gg - _g_off_g[:, None]) * _blk_d[_D][None, :]
    + _r_d[_D][None, :] * _wpad_d[_D][None, :]
    + _II[None, :]
)

_nc_cache = None


def _op_aps(xt, xo, ot, g_off, Gi, grp):
    """Build (out_ap, a_ap, b_ap) for one stair-step group."""
    src_odd, d0, n, w_pad, qoff = grp
    xt_full = xt[:]
    xo_full = xo[:]
    ot_full = ot[:]
    xlen = xt_full.ap[0][0]
    olen = ot_full.ap[0][0]
    a = AP(xt_full.tensor, g_off * C,
           [[xlen, P], [C, Gi], [0, n], [1, w_pad]])
    if src_odd:
        b = AP(xo_full.tensor, g_off * C + d0 - 1,
               [[xlen, P], [C, Gi], [2, n], [1, w_pad]])
    else:
        b = AP(xt_full.tensor, g_off * C + d0,
               [[xlen, P], [C, Gi], [2, n], [1, w_pad]])
    out = AP(ot_full.tensor, qoff * Gi,
             [[olen, P], [n * w_pad, Gi], [w_pad, n], [1, w_pad]])
    return out, a, b


def build_stair(g_iters=None, bufs: int = 3) -> bass.Bass:
    nc = bacc.Bacc(
        "TRN2",
        target_bir_lowering=False,
        debug=False,
        num_devices=N_CORES,
    )
    if g_iters is None:
        g_iters = G_ITERS
    assert sum(g_iters) == G
    g0 = g_iters[0]

    F8 = mybir.dt.float8e4
    x = nc.dram_tensor("x", [P, XLEN], BF, kind="ExternalInput")
    y = nc.dram_tensor("y", [P, G * K2B], BF, kind="ExternalOutput")
    y8 = nc.dram_tensor("y8", [P, G * K8], F8, kind="ExternalOutput")

    with tile.TileContext(nc) as tc:
        with (
            tc.tile_pool(name="xin", bufs=1) as xpool,
            tc.tile_pool(name="out", bufs=bufs) as opool,
            tc.tile_pool(name="out8", bufs=bufs) as o8pool,
        ):
            xt = xpool.tile([P, XLEN + XPAD], BF, tag="xt")
            xo = xpool.tile([P, XLEN + XPAD], BF, tag="xo")
            # Tail pads (read by the widest ops' spill columns).
            nc.gpsimd.memset(xt[:, XLEN : XLEN + XPAD], 0.0)
            nc.gpsimd.memset(xo[:, XLEN - 1 : XLEN + XPAD], 0.0)

            # Loads: chunk0 covers iter0's reads (incl. 4-elem spill into
            # the next group); bulk on the scalar ring in two chunks so
            # iter1 isn't gated on the full input.
            c0 = (g0 + 2) * C  # iter0 reads < (g0+1)*64 + 4
            gm = 16 * C
            nc.sync.dma_start(out=xt[:, 0:c0], in_=x[:, 0:c0])
            nc.sync.dma_start(out=xo[:, 0 : c0 - 1], in_=x[:, 1:c0])
            nc.scalar.dma_start(out=xt[:, c0:gm], in_=x[:, c0:gm])
            nc.scalar.dma_start(out=xt[:, gm:XLEN], in_=x[:, gm:XLEN])
            nc.scalar.dma_start(out=xo[:, c0 - 1 : gm - 1], in_=x[:, c0:gm])
            nc.scalar.dma_start(out=xo[:, gm - 1 : XLEN - 1], in_=x[:, gm:XLEN])

            g_off = 0
            for it, Gi in enumerate(g_iters):
                ot = opool.tile([P, Gi * K2], BF, tag="ot")
                ot8 = o8pool.tile([P, Gi * K8], F8, tag="ot8")
                for gi, grp in enumerate(PLAN):
                    out, a, b = _op_aps(xt, xo, ot, g_off, Gi, grp)
                    nc.vector.tensor_mul(out, a, b)
                    if gi == 6:
                        # [0:K2A) written: stream it on Q1 while the
                        # rest of the tile computes.
                        nc.sync.dma_start(
                            out=y[:, g_off * K2B : g_off * K2B + K2A * Gi],
                            in_=ot[:, 0 : K2A * Gi],
                        )
                    elif gi == 12:
                        # [K2A:K2B) written: stream on Q10.
                        nc.scalar.dma_start(
                            out=y[:, g_off * K2B + K2A * Gi
                                  : (g_off + Gi) * K2B],
                            in_=ot[:, K2A * Gi : K2B * Gi],
                        )
                    elif gi == 16:
                        # First fp8 cast + store while ops 17..21 run.
                        nc.scalar.copy(
                            ot8[:, 0 : K8M * Gi],
                            ot[:, K2B * Gi : (K2B + K8M) * Gi],
                        )
                        nc.scalar.dma_start(
                            out=y8[:, g_off * K8 : g_off * K8 + K8M * Gi],
                            in_=ot8[:, 0 : K8M * Gi],
                        )
                # Tail fp8 cast + store (trigger follows the cast in
                # scalar-engine program order).
                nc.scalar.copy(
                    ot8[:, K8M * Gi :], ot[:, (K2B + K8M) * Gi : K2 * Gi]
                )
                nc.scalar.dma_start(
                    out=y8[:, g_off * K8 + K8M * Gi : (g_off + Gi) * K8],
                    in_=ot8[:, K8M * Gi :],
                )
                g_off += Gi

    nc.finalize()
    return nc


def make_in_maps(x: np.ndarray) -> list[dict[str, np.ndarray]]:
    import ml_dtypes

    x = np.ascontiguousarray(x, dtype=np.float32).astype(ml_dtypes.bfloat16)
    return [
        {"x": x[c * BP : (c + 1) * BP].reshape(P, XLEN)} for c in range(N_CORES)
    ]


def kernel(**inputs: np.ndarray) -> np.ndarray:
    global _nc_cache
    if _nc_cache is None:
        _nc_cache = build_stair()
    res = run_bass_kernel_spmd(
        _nc_cache, make_in_maps(inputs["inputs"]), list(range(N_CORES))
    ).results
    import ml_dtypes

    def decode_core(c):
        yb = np.asarray(res[c]["y"])
        y8 = np.asarray(res[c]["y8"])
        if y8.dtype == np.uint8:
            y8 = y8.view(ml_dtypes.float8_e4m3fn)
        yb = yb.reshape(P, G * K2B).astype(np.float32)
        y8 = y8.reshape(P, G * K8).astype(np.float32)
        # Reassemble the virtual [P, G*K2] flat layout: per g-chunk the
        # blocks are in qoff order, bf16 blocks [0:K2B) then fp8 blocks.
        parts = []
        g_off = 0
        for Gi in G_ITERS:
            parts.append(yb[:, g_off * K2B : (g_off + Gi) * K2B])
            parts.append(y8[:, g_off * K8 : (g_off + Gi) * K8])
            g_off += Gi
        return np.concatenate(parts, axis=-1).reshape(BP, H, G * K2)

    ypad = np.concatenate([decode_core(c) for c in range(N_CORES)], axis=0)
    # Undo the stair-step block layout -> [W, triu (i,j)] + upcast.
    return np.take(ypad, _IDX, axis=-1)


# revision 36
# speedup vs baseline: 1.0963x; 1.0963x over previous
"""Triangular pairwise channel product on 8 Trainium2 NeuronCores.

out[b,h,w,k] = x[b,h,w,i_k] * x[b,h,w,j_k]  for the C*(C-1)/2 pairs
(i<j) in row-major (np.triu_indices) order.

Sharding: pure data parallel over batch - core c takes x[2c:2c+2].
Per core the 2*64*64 = 8192 spatial positions map to 128 SBUF
partitions (b_loc*64+h) x 64 groups (w).

d-offset formulation: for d in 1..63, prod_d[p,g,c] = x[p,g,c] *
x[p,g,c+d], c in [0, 64-d).  All operands are step-1 packed bf16 so
DVE's 2x_1p perf mode applies (2 elem/cycle).  Odd d reads its second
operand from x_odd (a one-channel-shifted DMA copy of x) to keep the
4B alignment 2x_1p requires.

v2 (stair-step): several consecutive same-parity d's share ONE
tensor_mul via a 4-level access pattern (partition, g, d, c): operand
a has d-stride 0 (broadcast), operand b d-stride 2 (overlapping
windows), out d-stride w_pad.  n=3 cuts DVE per-op overhead ~3x for
+6% rectangle-padding waste in SBUF/DMA bytes.  Measured baseline
facts: DVE tensor_tensor ~77ns/op marginal overhead, stores stream at
~433 GB/s/core on one HWDGE ring, ~7us fixed engine-barrier preamble,
~2.7us postamble.
"""

import numpy as np

import concourse.bacc as bacc
import concourse.bass as bass
import concourse.mybir as mybir
import concourse.tile as tile
from concourse.bass import AP
from concourse.bass_utils import run_bass_kernel_spmd

B, H, W, C = 16, 64, 64, 64
K = C * (C - 1) // 2  # 2016
N_CORES = 8
BP = B // N_CORES  # batch rows per core
P = BP * H         # 128 SBUF partitions
G = W              # position groups per partition
XLEN = G * C       # 4096 elements per partition
XPAD = 8           # tail pad: widest op row reads up to 4099
FP = mybir.dt.float32
BF = mybir.dt.bfloat16

# ---------------------------------------------------------------------------
# Stair-step plan: groups of n consecutive same-parity d's per DVE op.
# Group = (src_is_odd, d0, n, w_pad, qoff); row r handles d = d0 + 2r,
# covering c in [0, w_pad) (true width 64-d, rest is pad/waste).
# ---------------------------------------------------------------------------

def _make_plan(n: int = 3):
    """Evens take a leading n=4 group: [2,4,6,8] wastes the same 12
    channels as [2,4,6]+[8,..] padding would, but saves one op per
    tile."""
    plan = []
    qoff = 0
    for parity in (0, 1):  # even d's first (they don't need x_odd)
        if parity == 0:
            ds = [d for d in range(2, C, 2)]
            sizes = [4]
        else:
            ds = [d for d in range(1, C, 2)]
            sizes = []
        i = 0
        while i < len(ds):
            k = sizes.pop(0) if sizes else n
            grp = ds[i : i + k]
            w = C - grp[0]
            w_pad = w + (w % 2)
            plan.append((parity == 1, grp[0], len(grp), w_pad, qoff))
            qoff += len(grp) * w_pad
            i += k
    return plan, qoff


PLAN, K2 = _make_plan(3)
assert K2 == 2176, K2
N_OPS = len(PLAN)
assert N_OPS == 21, N_OPS

# GpSimd offload was tried and regressed badly (one op took 45 us and
# concurrent SBUF traffic slowed DVE ~14%): engines fight over SBUF
# bandwidth.  Keep everything on DVE.
GP_GROUPS: set[int] = set()

# Ramped: small first tiles prime the store pipeline early (kills the
# DMA bubble waiting on a big tile-1 compute); small last tile cuts the
# post-DVE tail (its cast + fp8 store + postamble end the kernel).
G_ITERS = [2, 4, 8, 12, 12, 12, 11, 3]
assert sum(G_ITERS) == G

_CUM = [0]
for _o, _d0, _n, _wp, _qo in PLAN:
    _CUM.append(_CUM[-1] + _n * _wp)

# fp8 split: blocks [K2B, K2) (the odd-parity d>=13 groups, ~35% of
# channels) are cast bf16->fp8_e4m3 on the idle scalar engine and
# stored at half the bytes.  Combined rel err ~1.6e-2 vs the 2e-2
# gate (bf16-only is 2.9e-3).  Cuts store bytes 17%: stores are the
# bottleneck in good runs and doubly so when a DMA engine runs
# degraded (~21% slower, environmental).  The LAST tile skips the
# casts (bf16 tail -> yt): its cast chain would serialize after the
# final multiply; the extra bytes are tiny.
_GI_A = 6   # ops 0.._GI_A cover [0:K2A)
_GI_B = next(i for i, p in enumerate(PLAN) if p[0] and p[1] == 13) - 1
_GI_C = next(i for i, p in enumerate(PLAN) if p[0] and p[1] == 37) - 1
K2A = 908
K2B = _CUM[_GI_B + 1]   # 1424
K8 = K2 - K2B           # 752
K8M = _CUM[_GI_C + 1] - K2B
assert _CUM[_GI_A + 1] >= K2A
LG = G_ITERS[-1]        # last tile groups (bf16 tail)
G8 = G - LG

# Host-side permutation: out[g, k] (triu pair k) -> flat device position.
# Device layout per g-iteration chunk (base g_off*K2): ops write contiguous
# blocks [qoff*Gi, (qoff + n*w_pad)*Gi), each block laid out (g, r, c).
_II, _JJ = np.triu_indices(C, k=1)
_qoff_d = np.zeros(C, dtype=np.int64)
_r_d = np.zeros(C, dtype=np.int64)
_wpad_d = np.zeros(C, dtype=np.int64)
_blk_d = np.zeros(C, dtype=np.int64)  # n*w_pad of d's group
for _odd, _d0, _n, _wp, _qo in PLAN:
    for _r in range(_n):
        _d = _d0 + 2 * _r
        _qoff_d[_d] = _qo
        _r_d[_d] = _r
        _wpad_d[_d] = _wp
        _blk_d[_d] = _n * _wp
_D = _JJ - _II  # [K]
_g_off_g = np.zeros(G, dtype=np.int64)
_Gi_g = np.zeros(G, dtype=np.int64)
_go = 0
for _Gi in G_ITERS:
    _g_off_g[_go : _go + _Gi] = _go
    _Gi_g[_go : _go + _Gi] = _Gi
    _go += _Gi
# IDX[g, k]: position in the [G*K2] flat per-partition output vector.
_gg = np.arange(G, dtype=np.int64)[:, None]
_IDX = (
    _g_off_g[:, None] * K2
    + _qoff_d[_D][None, :] * _Gi_g[:, None]
    + (_gg - _g_off_g[:, None]) * _blk_d[_D][None, :]
    + _r_d[_D][None, :] * _wpad_d[_D][None, :]
    + _II[None, :]
)

_nc_cache = None


def _op_aps(xt, xo, ot, g_off, Gi, grp):
    """Build (out_ap, a_ap, b_ap) for one stair-step group."""
    src_odd, d0, n, w_pad, qoff = grp
    xt_full = xt[:]
    xo_full = xo[:]
    ot_full = ot[:]
    xlen = xt_full.ap[0][0]
    olen = ot_full.ap[0][0]
    a = AP(xt_full.tensor, g_off * C,
           [[xlen, P], [C, Gi], [0, n], [1, w_pad]])
    if src_odd:
        b = AP(xo_full.tensor, g_off * C + d0 - 1,
               [[xlen, P], [C, Gi], [2, n], [1, w_pad]])
    else:
        b = AP(xt_full.tensor, g_off * C + d0,
               [[xlen, P], [C, Gi], [2, n], [1, w_pad]])
    out = AP(ot_full.tensor, qoff * Gi,
             [[olen, P], [n * w_pad, Gi], [w_pad, n], [1, w_pad]])
    return out, a, b


def build_stair(g_iters=None, bufs: int = 3) -> bass.Bass:
    nc = bacc.Bacc(
        "TRN2",
        target_bir_lowering=False,
        debug=False,
        num_devices=N_CORES,
    )
    if g_iters is None:
        g_iters = G_ITERS
    assert sum(g_iters) == G
    g0 = g_iters[0]

    F8 = mybir.dt.float8e4
    x = nc.dram_tensor("x", [P, XLEN], BF, kind="ExternalInput")
    y = nc.dram_tensor("y", [P, G * K2B], BF, kind="ExternalOutput")
    y8 = nc.dram_tensor("y8", [P, G8 * K8], F8, kind="ExternalOutput")
    yt = nc.dram_tensor("yt", [P, LG * K8], BF, kind="ExternalOutput")

    with tile.TileContext(nc) as tc:
        with (
            tc.tile_pool(name="xin", bufs=1) as xpool,
            tc.tile_pool(name="out", bufs=bufs) as opool,
            tc.tile_pool(name="out8", bufs=bufs) as o8pool,
        ):
            xt = xpool.tile([P, XLEN + XPAD], BF, tag="xt")
            xo = xpool.tile([P, XLEN + XPAD], BF, tag="xo")
            # Tail pads (read by the widest ops' spill columns).
            nc.gpsimd.memset(xt[:, XLEN : XLEN + XPAD], 0.0)
            nc.gpsimd.memset(xo[:, XLEN - 1 : XLEN + XPAD], 0.0)

            # Loads: chunk0 covers iter0's reads (incl. 4-elem spill into
            # the next group); bulk on the scalar ring in two chunks so
            # iter1 isn't gated on the full input.
            c0 = (g0 + 2) * C  # iter0 reads < (g0+1)*64 + 4
            gm = 16 * C
            nc.sync.dma_start(out=xt[:, 0:c0], in_=x[:, 0:c0])
            nc.sync.dma_start(out=xo[:, 0 : c0 - 1], in_=x[:, 1:c0])
            nc.scalar.dma_start(out=xt[:, c0:gm], in_=x[:, c0:gm])
            nc.scalar.dma_start(out=xt[:, gm:XLEN], in_=x[:, gm:XLEN])
            nc.scalar.dma_start(out=xo[:, c0 - 1 : gm - 1], in_=x[:, c0:gm])
            nc.scalar.dma_start(out=xo[:, gm - 1 : XLEN - 1], in_=x[:, gm:XLEN])

            g_off = 0
            for it, Gi in enumerate(g_iters):
                last = it == len(g_iters) - 1
                ot = opool.tile([P, Gi * K2], BF, tag="ot")
                if not last:
                    ot8 = o8pool.tile([P, Gi * K8], F8, tag="ot8")
                for gi, grp in enumerate(PLAN):
                    out, a, b = _op_aps(xt, xo, ot, g_off, Gi, grp)
                    nc.vector.tensor_mul(out, a, b)
                    if gi == _GI_A:
                        # [0:K2A) written: stream it on Q1 while the
                        # rest of the tile computes.
                        nc.sync.dma_start(
                            out=y[:, g_off * K2B : g_off * K2B + K2A * Gi],
                            in_=ot[:, 0 : K2A * Gi],
                        )
                    elif gi == _GI_B:
                        # [K2A:K2B) written: stream on Q10.
                        nc.scalar.dma_start(
                            out=y[:, g_off * K2B + K2A * Gi
                                  : (g_off + Gi) * K2B],
                            in_=ot[:, K2A * Gi : K2B * Gi],
                        )
                    elif gi == _GI_C and not last:
                        # First fp8 cast + store while the last ops run.
                        nc.scalar.copy(
                            ot8[:, 0 : K8M * Gi],
                            ot[:, K2B * Gi : (K2B + K8M) * Gi],
                        )
                        nc.scalar.dma_start(
                            out=y8[:, g_off * K8 : g_off * K8 + K8M * Gi],
                            in_=ot8[:, 0 : K8M * Gi],
                        )
                if not last:
                    # Tail fp8 cast + store (trigger follows the cast
                    # in scalar-engine program order).
                    nc.scalar.copy(
                        ot8[:, K8M * Gi :],
                        ot[:, (K2B + K8M) * Gi : K2 * Gi],
                    )
                    nc.scalar.dma_start(
                        out=y8[:, g_off * K8 + K8M * Gi
                              : (g_off + Gi) * K8],
                        in_=ot8[:, K8M * Gi :],
                    )
                else:
                    # Last tile: skip the cast, store the tail as bf16.
                    nc.sync.dma_start(
                        out=yt[:], in_=ot[:, K2B * Gi : K2 * Gi]
                    )
                g_off += Gi

    nc.finalize()
    return nc


def make_in_maps(x: np.ndarray) -> list[dict[str, np.ndarray]]:
    import ml_dtypes

    x = np.ascontiguousarray(x, dtype=np.float32).astype(ml_dtypes.bfloat16)
    return [
        {"x": x[c * BP : (c + 1) * BP].reshape(P, XLEN)} for c in range(N_CORES)
    ]


def kernel(**inputs: np.ndarray) -> np.ndarray:
    global _nc_cache
    if _nc_cache is None:
        _nc_cache = build_stair()
    res = run_bass_kernel_spmd(
        _nc_cache, make_in_maps(inputs["inputs"]), list(range(N_CORES))
    ).results
    import ml_dtypes

    def decode_core(c):
        yb = np.asarray(res[c]["y"])
        y8 = np.asarray(res[c]["y8"])
        if y8.dtype == np.uint8:
            y8 = y8.view(ml_dtypes.float8_e4m3fn)
        yb = yb.reshape(P, G * K2B).astype(np.float32)
        y8 = y8.reshape(P, G8 * K8).astype(np.float32)
        ytl = np.asarray(res[c]["yt"]).reshape(P, LG * K8).astype(np.float32)
        # Reassemble the virtual [P, G*K2] flat layout: per g-chunk the
        # blocks are in qoff order, bf16 blocks [0:K2B) then the tail
        # blocks (fp8, except bf16 for the last chunk).
        parts = []
        g_off = 0
        for it, Gi in enumerate(G_ITERS):
            parts.append(yb[:, g_off * K2B : (g_off + Gi) * K2B])
            if it == len(G_ITERS) - 1:
                parts.append(ytl)
            else:
                parts.append(y8[:, g_off * K8 : (g_off + Gi) * K8])
            g_off += Gi
        return np.concatenate(parts, axis=-1).reshape(BP, H, G * K2)

    ypad = np.concatenate([decode_core(c) for c in range(N_CORES)], axis=0)
    # Undo the stair-step block layout -> [W, triu (i,j)] + upcast.
    return np.take(ypad, _IDX, axis=-1)
